# revision 15
# baseline (speedup 1.0000x reference)
"""Chamfer distance kernel for Trainium2 (8 NeuronCores, batch-parallel).

Strategy
--------
B=8 batches, one per core (SPMD: same program, per-core data).
Per core (N=M=8192, 3-D points):
  d[n,m] = |x1_n|^2 + |x2_m|^2 - 2 x1_n.x2_m  is computed fully inside
  PSUM by a single matmul over homogeneous-coordinate rows.  In the
  default "f16x2" mode every f32 operand row is split into an fp16
  (hi, lo) pair and the dot product expands into K=16 fp16 rows
  (4 per coordinate + 4 norm/ones rows): fp16 matmuls run 4x faster
  than f32 on the PE while the hi/lo split keeps ~22 mantissa bits, so
  the argmin matches the f32 reference.
  Two symmetric passes (rows = x1 points, then rows = x2 points) give
  the row-direction min+argmin for both outputs.  Per 128-row block the
  ACT engine moves PSUM->SBUF, DVE computes the row min with ONE
  min/min tensor_tensor_scan over the two row halves (stride-0 out
  lands the final state straight in the dist column) and one full-row
  max_index (first occurrence == np.argmin tie-break) for the index.
Outputs come back as [128, 64] tiles (partition-major), the host
transpose-flattens them.
"""

import numpy as np

import concourse.bacc as bacc
import concourse.bass as bass
import concourse.mybir as mybir
from concourse import tile
from concourse.bass_utils import run_bass_kernel_spmd

F32 = mybir.dt.float32
F16 = mybir.dt.float16
I32 = mybir.dt.int32
U32 = mybir.dt.uint32

_PROGRAM_CACHE = {}

# staging-row indices for the fp16 split pieces
_T_XH, _T_YH, _T_ZH = 0, 1, 2        # hi(x, y, z)
_T_XL, _T_YL, _T_ZL = 3, 4, 5        # lo(x, y, z)
_T_M2XH, _T_M2YH, _T_M2ZH = 6, 7, 8  # -2*hi
_T_M2XL, _T_M2YL, _T_M2ZL = 9, 10, 11  # -2*lo
_T_NH, _T_NL, _T_ONE = 12, 13, 14    # hi(n), lo(n), ones

# K=16 row contents of the two matmul operand forms, as staging indices.
# Pairing k: A_k . B_k summed over k gives n + n' - 2 x.x'.
_A_ROWS = [_T_M2XH, _T_M2XH, _T_M2XL, _T_M2XL,
           _T_M2YH, _T_M2YH, _T_M2YL, _T_M2YL,
           _T_M2ZH, _T_M2ZH, _T_M2ZL, _T_M2ZL,
           _T_ONE, _T_ONE, _T_NH, _T_NL]
_B_ROWS = [_T_XH, _T_XL, _T_XH, _T_XL,
           _T_YH, _T_YL, _T_YH, _T_YL,
           _T_ZH, _T_ZL, _T_ZH, _T_ZL,
           _T_NH, _T_NL, _T_ONE, _T_ONE]


def _emit_pass(nc, lhsU, rhsU, base, krows, dtile, itile, rowbuf_pool,
               psum_pool, scratch_pool, n_pts, m_pts, mm_dt):
    """One direction: for each 128-row block of lhs points, min+argmin over
    all m_pts columns.  lhsU[base:base+krows] is the stationary operand
    (A-form), rhsU[base:base+krows] the moving one (B-form); matmul
    requires both operands at the same partition base."""
    n_blocks = n_pts // 128
    n_groups = m_pts // 2048
    half = m_pts // 2

    for nb in range(n_blocks):
        rowbuf = rowbuf_pool.tile([128, m_pts], F32, tag="rowbuf")
        lhs_ap = lhsU[base:base + krows, nb * 128:(nb + 1) * 128]
        if mm_dt is not None:
            lhs_ap = lhs_ap.bitcast(mm_dt)
        for g in range(n_groups):
            psum = psum_pool.tile([128, 2048], F32, tag="psum")
            for q in range(4):
                rhs_ap = rhsU[base:base + krows,
                              (g * 4 + q) * 512:(g * 4 + q + 1) * 512]
                if mm_dt is not None:
                    rhs_ap = rhs_ap.bitcast(mm_dt)
                nc.tensor.matmul(
                    psum[:, q * 512:(q + 1) * 512],
                    lhs_ap,
                    rhs_ap,
                    start=True, stop=True,
                )
            nc.scalar.activation(
                rowbuf[:, g * 2048:(g + 1) * 2048], psum[:],
                mybir.ActivationFunctionType.Copy,
            )
        # row min -> dist column: min/min scan over both halves at once
        nc.vector.tensor_tensor_scan(
            dtile[:, nb:nb + 1].broadcast_to((128, half)),
            rowbuf[:, 0:half], rowbuf[:, half:],
            initial=3.0e38,
            op0=mybir.AluOpType.min, op1=mybir.AluOpType.min,
        )
        # match the min value back to its first position
        q8 = scratch_pool.tile([128, 8], F32, tag="q8")
        ix = scratch_pool.tile([128, 8], U32, tag="ix")
        nc.vector.tensor_copy(q8[:], dtile[:, nb:nb + 1].broadcast_to((128, 8)))
        nc.vector.max_index(ix[:], q8[:], rowbuf[:])
        nc.vector.tensor_copy(itile[:, nb:nb + 1], ix[:, 0:1])


def _emit_prep_f16x2(nc, tc, U1, U2, c1, c2, a1, b1, a2, b2, n_pts, m_pts):
    """Load the host-prepared fp16 hi/lo coordinate forms and fill in the
    norm rows on device.  U1: A-form at base 0, B-form at base 32.  U2:
    B-form at 0, A-form at 32 (pass A pairs U1[0:16] x U2[0:16]; pass B
    pairs U2[32:48] x U1[32:48]).  Norm rows: A-form 14/15 = hi/lo(n),
    B-form 12/13 = hi/lo(n), computed from the f32 coords via the
    ones-matmul and an exact fp16 hi/lo split."""
    with tc.tile_pool(name="prep", bufs=1) as prep, \
         tc.tile_pool(name="preppsum", bufs=2, space="PSUM") as ppsum:
        ones_col = prep.tile([3, 1], F32, tag="ones_col")
        nc.vector.memset(ones_col[:], 1.0)
        for U, c, af, bf, npts, a, b in (
                (U1, c1, a1, b1, n_pts, 0, 32),
                (U2, c2, a2, b2, m_pts, 32, 0)):
            nc.sync.dma_start(U[a:a + 16, :], af.ap())
            nc.sync.dma_start(U[b:b + 16, :], bf.ap())
            S32 = prep.tile([3, npts], F32, tag="S32")
            SQ = prep.tile([3, npts], F32, tag="SQ")
            nrow = prep.tile([1, npts], F32, tag="nrow")
            NH32 = prep.tile([1, npts], F32, tag="NH32")
            NH = prep.tile([1, npts], F16, tag="NH")
            NL = prep.tile([1, npts], F16, tag="NL")

            nc.sync.dma_start(S32[:], c.ap()[0:3, :])  # x, y, z
            # norms in exact f32: n = x^2 + y^2 + z^2 via ones-matmul
            nc.scalar.activation(SQ[:], S32[:],
                                 mybir.ActivationFunctionType.Square)
            for cc in range(npts // 512):
                ps = ppsum.tile([1, 512], F32, tag="ps")
                nc.tensor.matmul(ps[:], ones_col[:],
                                 SQ[:, cc * 512:(cc + 1) * 512],
                                 start=True, stop=True)
                nc.scalar.activation(nrow[:, cc * 512:(cc + 1) * 512],
                                     ps[:], mybir.ActivationFunctionType.Copy)
            # exact hi/lo split of n: hi = fp16(n); lo = fp16(n - f32(hi))
            nc.scalar.activation(NH[:], nrow[:],
                                 mybir.ActivationFunctionType.Copy)
            nc.scalar.activation(NH32[:], NH[:],
                                 mybir.ActivationFunctionType.Copy)
            nc.vector.tensor_tensor(nrow[:], nrow[:], NH32[:],
                                    mybir.AluOpType.subtract)
            nc.scalar.activation(NL[:], nrow[:],
                                 mybir.ActivationFunctionType.Copy)
            for row, src in ((a + 14, NH), (a + 15, NL),
                             (b + 12, NH), (b + 13, NL)):
                nc.sync.dma_start(U[row:row + 1, :], src[:])


def _build_program(n_pts=8192, m_pts=8192, n_cores=8, mm_dtype="f16x2",
                   repeat=1):
    key = (n_pts, m_pts, n_cores, mm_dtype, repeat)
    if key in _PROGRAM_CACHE:
        return _PROGRAM_CACHE[key]

    nc = bacc.Bacc("TRN2", target_bir_lowering=False, debug=False,
                   num_devices=n_cores)
    # rows [x, y, z, ones]; ones seeds the homogeneous-coordinate rows
    c1 = nc.dram_tensor("c1", [4, n_pts], F32, kind="ExternalInput")
    c2 = nc.dram_tensor("c2", [4, m_pts], F32, kind="ExternalInput")
    if mm_dtype == "f16x2":
        a1 = nc.dram_tensor("a1", [16, n_pts], F16, kind="ExternalInput")
        b1 = nc.dram_tensor("b1", [16, n_pts], F16, kind="ExternalInput")
        a2 = nc.dram_tensor("a2", [16, m_pts], F16, kind="ExternalInput")
        b2 = nc.dram_tensor("b2", [16, m_pts], F16, kind="ExternalInput")
    d1 = nc.dram_tensor("d1", [128, n_pts // 128], F32, kind="ExternalOutput")
    i1 = nc.dram_tensor("i1", [128, n_pts // 128], I32, kind="ExternalOutput")
    d2 = nc.dram_tensor("d2", [128, m_pts // 128], F32, kind="ExternalOutput")
    i2 = nc.dram_tensor("i2", [128, m_pts // 128], I32, kind="ExternalOutput")

    with tile.TileContext(nc) as tc:
        with tc.tile_pool(name="persist", bufs=1) as persist, \
             tc.tile_pool(name="finals", bufs=1) as finals:
            d1t = finals.tile([128, n_pts // 128], F32, tag="d1t")
            i1t = finals.tile([128, n_pts // 128], I32, tag="i1t")
            d2t = finals.tile([128, m_pts // 128], F32, tag="d2t")
            i2t = finals.tile([128, m_pts // 128], I32, tag="i2t")

            if mm_dtype == "f16x2":
                krows, mm_dt = 16, None
                U1 = persist.tile([48, n_pts], F16, tag="U1")
                U2 = persist.tile([48, m_pts], F16, tag="U2")
                _emit_prep_f16x2(nc, tc, U1, U2, c1, c2, a1, b1, a2, b2,
                                 n_pts, m_pts)
            else:
                # exact-f32 fallback: K=5 homogeneous rows
                # U1: lhsT-form (A = [-2x,-2y,-2z, 1, n1]) at rows 0-4,
                #     rhs-form  (B = [x, y, z, n1, 1])     at rows 32-36
                # U2: rhs-form at rows 0-4, lhsT-form at rows 32-36
                krows, mm_dt = 5, None
                U1 = persist.tile([37, n_pts], F32, tag="U1")
                U2 = persist.tile([37, m_pts], F32, tag="U2")
                ones_col = persist.tile([35, 1], F32, tag="ones_col")
                nc.vector.memset(ones_col[:], 1.0)
                with tc.tile_pool(name="prep", bufs=1) as prep, \
                     tc.tile_pool(name="preppsum", bufs=2, space="PSUM") as ppsum:
                    for U, c, npts, a, b in ((U1, c1, n_pts, 0, 32),
                                             (U2, c2, m_pts, 32, 0)):
                        nc.sync.dma_start(U[b:b + 3, :], c.ap()[0:3, :])
                        nc.sync.dma_start(U[a + 3:a + 4, :], c.ap()[3:4, :])
                        nc.sync.dma_start(U[b + 4:b + 5, :], c.ap()[3:4, :])
                        sq = prep.tile([35, npts], F32, tag="sq")
                        nrow = prep.tile([1, npts], F32, tag="nrow")
                        nc.scalar.activation(sq[b:b + 3, :], U[b:b + 3, :],
                                             mybir.ActivationFunctionType.Square)
                        for cc in range(npts // 512):
                            ps = ppsum.tile([1, 512], F32, tag="ps")
                            nc.tensor.matmul(ps[:], ones_col[b:b + 3, :],
                                             sq[b:b + 3, cc * 512:(cc + 1) * 512],
                                             start=True, stop=True)
                            nc.scalar.activation(
                                nrow[:, cc * 512:(cc + 1) * 512],
                                ps[:], mybir.ActivationFunctionType.Copy)
                        nc.sync.dma_start(U[b + 3:b + 4, :], nrow[:])
                        nc.sync.dma_start(U[a + 4:a + 5, :], nrow[:])
                        nc.vector.tensor_scalar(
                            out=sq[b:b + 3, :], in0=U[b:b + 3, :],
                            scalar1=-2.0, scalar2=None,
                            op0=mybir.AluOpType.mult)
                        nc.sync.dma_start(U[a:a + 3, :], sq[b:b + 3, :])

            # ---- main passes ----
            with tc.tile_pool(name="rowbuf", bufs=2) as rowbuf_pool, \
                 tc.tile_pool(name="mainpsum", bufs=2, space="PSUM") as psum_pool, \
                 tc.tile_pool(name="scratch", bufs=2) as scratch_pool:
                for _ in range(repeat):
                    _emit_pass(nc, U1, U2, 0, krows, d1t, i1t, rowbuf_pool,
                               psum_pool, scratch_pool, n_pts, m_pts, mm_dt)
                    _emit_pass(nc, U2, U1, 32, krows, d2t, i2t, rowbuf_pool,
                               psum_pool, scratch_pool, m_pts, n_pts, mm_dt)

            # clamp tiny negative rounding like the reference's max(d, 0)
            nc.scalar.activation(d1t[:], d1t[:], mybir.ActivationFunctionType.Relu)
            nc.scalar.activation(d2t[:], d2t[:], mybir.ActivationFunctionType.Relu)
            nc.sync.dma_start(d1.ap(), d1t[:])
            nc.sync.dma_start(i1.ap(), i1t[:])
            nc.sync.dma_start(d2.ap(), d2t[:])
            nc.sync.dma_start(i2.ap(), i2t[:])

    nc.compile()
    _PROGRAM_CACHE[key] = nc
    return nc


def _host_forms(xyz):
    """fp16 hi/lo split of the [N, 3] coords, laid out as the two 16-row
    matmul operand forms (norm rows zero-filled; the device computes and
    writes them)."""
    n = xyz.shape[0]
    co = np.ascontiguousarray(xyz.T)  # [3, n] f32
    hi = co.astype(np.float16)
    lo = (co - hi.astype(np.float32)).astype(np.float16)
    m2hi = (-2.0 * hi.astype(np.float32)).astype(np.float16)
    m2lo = (-2.0 * lo.astype(np.float32)).astype(np.float16)
    zero = np.zeros((1, n), np.float16)
    one = np.ones((1, n), np.float16)
    # staging piece list indexed by the _T_* constants
    P = [hi[0:1], hi[1:2], hi[2:3], lo[0:1], lo[1:2], lo[2:3],
         m2hi[0:1], m2hi[1:2], m2hi[2:3], m2lo[0:1], m2lo[1:2], m2lo[2:3],
         zero, zero, one]
    A = np.concatenate([P[t] for t in _A_ROWS], axis=0)
    Bf = np.concatenate([P[t] for t in _B_ROWS], axis=0)
    return A, Bf


def make_in_maps(xyz1, xyz2, mm_dtype="f16x2"):
    B, N, _ = xyz1.shape
    M = xyz2.shape[1]
    ones_n = np.ones((1, N), np.float32)
    ones_m = np.ones((1, M), np.float32)
    in_maps = []
    for b in range(B):
        m = {"c1": np.concatenate([np.ascontiguousarray(xyz1[b].T), ones_n]),
             "c2": np.concatenate([np.ascontiguousarray(xyz2[b].T), ones_m])}
        if mm_dtype == "f16x2":
            m["a1"], m["b1"] = _host_forms(xyz1[b])
            m["a2"], m["b2"] = _host_forms(xyz2[b])
        in_maps.append(m)
    return in_maps


def kernel(xyz1: np.ndarray, xyz2: np.ndarray, mm_dtype: str = "f16x2",
           repeat: int = 1, _return_results_only: bool = False):
    xyz1 = np.asarray(xyz1, dtype=np.float32)
    xyz2 = np.asarray(xyz2, dtype=np.float32)
    B, N, _ = xyz1.shape
    _, M, _ = xyz2.shape
    assert B == 8 and N == 8192 and M == 8192, (B, N, M)

    nc = _build_program(N, M, B, mm_dtype, repeat)
    in_maps = make_in_maps(xyz1, xyz2, mm_dtype)
    res = run_bass_kernel_spmd(nc, in_maps, list(range(B)))

    dist1 = np.empty((B, N), np.float32)
    dist2 = np.empty((B, M), np.float32)
    idx1 = np.empty((B, N), np.int32)
    idx2 = np.empty((B, M), np.int32)
    for b in range(B):
        r = res.results[b]
        dist1[b] = np.asarray(r["d1"]).T.reshape(-1)
        idx1[b] = np.asarray(r["i1"]).T.reshape(-1)
        dist2[b] = np.asarray(r["d2"]).T.reshape(-1)
        idx2[b] = np.asarray(r["i2"]).T.reshape(-1)
    return dist1, dist2, idx1, idx2


# revision 16
# speedup vs baseline: 1.9235x; 1.9235x over previous
"""Chamfer distance kernel for Trainium2 (8 NeuronCores, batch-parallel).

Strategy
--------
B=8 batches, one per core (SPMD: same program, per-core data).
Per core (N=M=8192, 3-D points):
  d[n,m] = |x1_n|^2 + |x2_m|^2 - 2 x1_n.x2_m  is computed fully inside
  PSUM by a single matmul over homogeneous-coordinate rows.  In the
  default "f16x2" mode every f32 operand row is split into an fp16
  (hi, lo) pair and the dot product expands into K=16 fp16 rows
  (4 per coordinate + 4 norm/ones rows): fp16 matmuls run 4x faster
  than f32 on the PE while the hi/lo split keeps ~22 mantissa bits, so
  the argmin matches the f32 reference.
  Two symmetric passes (rows = x1 points, then rows = x2 points) give
  the row-direction min+argmin for both outputs.  Per 128-row block the
  ACT engine moves PSUM->SBUF, DVE computes the row min with ONE
  min/min tensor_tensor_scan over the two row halves (stride-0 out
  lands the final state straight in the dist column) and one full-row
  max_index (first occurrence == np.argmin tie-break) for the index.
Outputs come back as [128, 64] tiles (partition-major), the host
transpose-flattens them.
"""

import numpy as np

import concourse.bacc as bacc
import concourse.bass as bass
import concourse.mybir as mybir
from concourse import tile
from concourse.bass_utils import run_bass_kernel_spmd

F32 = mybir.dt.float32
F16 = mybir.dt.float16
I32 = mybir.dt.int32
U32 = mybir.dt.uint32

_PROGRAM_CACHE = {}

# staging-row indices for the fp16 split pieces
_T_XH, _T_YH, _T_ZH = 0, 1, 2        # hi(x, y, z)
_T_XL, _T_YL, _T_ZL = 3, 4, 5        # lo(x, y, z)
_T_M2XH, _T_M2YH, _T_M2ZH = 6, 7, 8  # -2*hi
_T_M2XL, _T_M2YL, _T_M2ZL = 9, 10, 11  # -2*lo
_T_NH, _T_NL, _T_ONE = 12, 13, 14    # hi(n), lo(n), ones

# K=16 row contents of the two matmul operand forms, as staging indices.
# Pairing k: A_k . B_k summed over k gives n + n' - 2 x.x'.
_A_ROWS = [_T_M2XH, _T_M2XH, _T_M2XL, _T_M2XL,
           _T_M2YH, _T_M2YH, _T_M2YL, _T_M2YL,
           _T_M2ZH, _T_M2ZH, _T_M2ZL, _T_M2ZL,
           _T_ONE, _T_ONE, _T_NH, _T_NL]
_B_ROWS = [_T_XH, _T_XL, _T_XH, _T_XL,
           _T_YH, _T_YL, _T_YH, _T_YL,
           _T_ZH, _T_ZL, _T_ZH, _T_ZL,
           _T_NH, _T_NL, _T_ONE, _T_ONE]


def _emit_pass(nc, lhsU, rhsU, base, krows, dtile, itile, rowbuf_pool,
               psum_pool, scratch_pool, n_pts, m_pts, mm_dt):
    """One direction: for each 128-row block of lhs points, min+argmin over
    all m_pts columns.  lhsU[base:base+krows] is the stationary operand
    (A-form), rhsU[base:base+krows] the moving one (B-form); matmul
    requires both operands at the same partition base."""
    n_blocks = n_pts // 128
    n_groups = m_pts // 2048
    half = m_pts // 2

    for nb in range(n_blocks):
        rowbuf = rowbuf_pool.tile([128, m_pts], F32, tag="rowbuf")
        lhs_ap = lhsU[base:base + krows, nb * 128:(nb + 1) * 128]
        if mm_dt is not None:
            lhs_ap = lhs_ap.bitcast(mm_dt)
        for g in range(n_groups):
            psum = psum_pool.tile([128, 2048], F32, tag="psum")
            for q in range(4):
                rhs_ap = rhsU[base:base + krows,
                              (g * 4 + q) * 512:(g * 4 + q + 1) * 512]
                if mm_dt is not None:
                    rhs_ap = rhs_ap.bitcast(mm_dt)
                nc.tensor.matmul(
                    psum[:, q * 512:(q + 1) * 512],
                    lhs_ap,
                    rhs_ap,
                    start=True, stop=True,
                )
            nc.scalar.activation(
                rowbuf[:, g * 2048:(g + 1) * 2048], psum[:],
                mybir.ActivationFunctionType.Copy,
            )
        # row min -> dist column (full-row reduce; on HW this runs at the
        # same fast DVE rate as max_index, unlike the serialized scan)
        nc.vector.tensor_reduce(
            dtile[:, nb:nb + 1], rowbuf[:],
            axis=mybir.AxisListType.X, op=mybir.AluOpType.min,
        )
        # match the min value back to its first position
        q8 = scratch_pool.tile([128, 8], F32, tag="q8")
        ix = scratch_pool.tile([128, 8], U32, tag="ix")
        nc.vector.tensor_copy(q8[:], dtile[:, nb:nb + 1].broadcast_to((128, 8)))
        nc.vector.max_index(ix[:], q8[:], rowbuf[:])
        nc.vector.tensor_copy(itile[:, nb:nb + 1], ix[:, 0:1])


def _emit_prep_f16x2(nc, tc, U1, U2, c1, c2, a1, b1, a2, b2, n_pts, m_pts):
    """Load the host-prepared fp16 hi/lo coordinate forms and fill in the
    norm rows on device.  U1: A-form at base 0, B-form at base 32.  U2:
    B-form at 0, A-form at 32 (pass A pairs U1[0:16] x U2[0:16]; pass B
    pairs U2[32:48] x U1[32:48]).  Norm rows: A-form 14/15 = hi/lo(n),
    B-form 12/13 = hi/lo(n), computed from the f32 coords via the
    ones-matmul and an exact fp16 hi/lo split."""
    with tc.tile_pool(name="prep", bufs=1) as prep, \
         tc.tile_pool(name="preppsum", bufs=2, space="PSUM") as ppsum:
        ones_col = prep.tile([3, 1], F32, tag="ones_col")
        nc.vector.memset(ones_col[:], 1.0)
        for U, c, af, bf, npts, a, b in (
                (U1, c1, a1, b1, n_pts, 0, 32),
                (U2, c2, a2, b2, m_pts, 32, 0)):
            nc.sync.dma_start(U[a:a + 16, :], af.ap())
            nc.sync.dma_start(U[b:b + 16, :], bf.ap())
            S32 = prep.tile([3, npts], F32, tag="S32")
            SQ = prep.tile([3, npts], F32, tag="SQ")
            nrow = prep.tile([1, npts], F32, tag="nrow")
            NH32 = prep.tile([1, npts], F32, tag="NH32")
            NH = prep.tile([1, npts], F16, tag="NH")
            NL = prep.tile([1, npts], F16, tag="NL")

            nc.sync.dma_start(S32[:], c.ap()[0:3, :])  # x, y, z
            # norms in exact f32: n = x^2 + y^2 + z^2 via ones-matmul
            nc.scalar.activation(SQ[:], S32[:],
                                 mybir.ActivationFunctionType.Square)
            for cc in range(npts // 512):
                ps = ppsum.tile([1, 512], F32, tag="ps")
                nc.tensor.matmul(ps[:], ones_col[:],
                                 SQ[:, cc * 512:(cc + 1) * 512],
                                 start=True, stop=True)
                nc.scalar.activation(nrow[:, cc * 512:(cc + 1) * 512],
                                     ps[:], mybir.ActivationFunctionType.Copy)
            # exact hi/lo split of n: hi = fp16(n); lo = fp16(n - f32(hi))
            nc.scalar.activation(NH[:], nrow[:],
                                 mybir.ActivationFunctionType.Copy)
            nc.scalar.activation(NH32[:], NH[:],
                                 mybir.ActivationFunctionType.Copy)
            nc.vector.tensor_tensor(nrow[:], nrow[:], NH32[:],
                                    mybir.AluOpType.subtract)
            nc.scalar.activation(NL[:], nrow[:],
                                 mybir.ActivationFunctionType.Copy)
            for row, src in ((a + 14, NH), (a + 15, NL),
                             (b + 12, NH), (b + 13, NL)):
                nc.sync.dma_start(U[row:row + 1, :], src[:])


def _build_program(n_pts=8192, m_pts=8192, n_cores=8, mm_dtype="f16x2",
                   repeat=1):
    key = (n_pts, m_pts, n_cores, mm_dtype, repeat)
    if key in _PROGRAM_CACHE:
        return _PROGRAM_CACHE[key]

    nc = bacc.Bacc("TRN2", target_bir_lowering=False, debug=False,
                   num_devices=n_cores)
    # rows [x, y, z, ones]; ones seeds the homogeneous-coordinate rows
    c1 = nc.dram_tensor("c1", [4, n_pts], F32, kind="ExternalInput")
    c2 = nc.dram_tensor("c2", [4, m_pts], F32, kind="ExternalInput")
    if mm_dtype == "f16x2":
        a1 = nc.dram_tensor("a1", [16, n_pts], F16, kind="ExternalInput")
        b1 = nc.dram_tensor("b1", [16, n_pts], F16, kind="ExternalInput")
        a2 = nc.dram_tensor("a2", [16, m_pts], F16, kind="ExternalInput")
        b2 = nc.dram_tensor("b2", [16, m_pts], F16, kind="ExternalInput")
    d1 = nc.dram_tensor("d1", [128, n_pts // 128], F32, kind="ExternalOutput")
    i1 = nc.dram_tensor("i1", [128, n_pts // 128], I32, kind="ExternalOutput")
    d2 = nc.dram_tensor("d2", [128, m_pts // 128], F32, kind="ExternalOutput")
    i2 = nc.dram_tensor("i2", [128, m_pts // 128], I32, kind="ExternalOutput")

    with tile.TileContext(nc) as tc:
        with tc.tile_pool(name="persist", bufs=1) as persist, \
             tc.tile_pool(name="finals", bufs=1) as finals:
            d1t = finals.tile([128, n_pts // 128], F32, tag="d1t")
            i1t = finals.tile([128, n_pts // 128], I32, tag="i1t")
            d2t = finals.tile([128, m_pts // 128], F32, tag="d2t")
            i2t = finals.tile([128, m_pts // 128], I32, tag="i2t")

            if mm_dtype == "f16x2":
                krows, mm_dt = 16, None
                U1 = persist.tile([48, n_pts], F16, tag="U1")
                U2 = persist.tile([48, m_pts], F16, tag="U2")
                _emit_prep_f16x2(nc, tc, U1, U2, c1, c2, a1, b1, a2, b2,
                                 n_pts, m_pts)
            else:
                # exact-f32 fallback: K=5 homogeneous rows
                # U1: lhsT-form (A = [-2x,-2y,-2z, 1, n1]) at rows 0-4,
                #     rhs-form  (B = [x, y, z, n1, 1])     at rows 32-36
                # U2: rhs-form at rows 0-4, lhsT-form at rows 32-36
                krows, mm_dt = 5, None
                U1 = persist.tile([37, n_pts], F32, tag="U1")
                U2 = persist.tile([37, m_pts], F32, tag="U2")
                ones_col = persist.tile([35, 1], F32, tag="ones_col")
                nc.vector.memset(ones_col[:], 1.0)
                with tc.tile_pool(name="prep", bufs=1) as prep, \
                     tc.tile_pool(name="preppsum", bufs=2, space="PSUM") as ppsum:
                    for U, c, npts, a, b in ((U1, c1, n_pts, 0, 32),
                                             (U2, c2, m_pts, 32, 0)):
                        nc.sync.dma_start(U[b:b + 3, :], c.ap()[0:3, :])
                        nc.sync.dma_start(U[a + 3:a + 4, :], c.ap()[3:4, :])
                        nc.sync.dma_start(U[b + 4:b + 5, :], c.ap()[3:4, :])
                        sq = prep.tile([35, npts], F32, tag="sq")
                        nrow = prep.tile([1, npts], F32, tag="nrow")
                        nc.scalar.activation(sq[b:b + 3, :], U[b:b + 3, :],
                                             mybir.ActivationFunctionType.Square)
                        for cc in range(npts // 512):
                            ps = ppsum.tile([1, 512], F32, tag="ps")
                            nc.tensor.matmul(ps[:], ones_col[b:b + 3, :],
                                             sq[b:b + 3, cc * 512:(cc + 1) * 512],
                                             start=True, stop=True)
                            nc.scalar.activation(
                                nrow[:, cc * 512:(cc + 1) * 512],
                                ps[:], mybir.ActivationFunctionType.Copy)
                        nc.sync.dma_start(U[b + 3:b + 4, :], nrow[:])
                        nc.sync.dma_start(U[a + 4:a + 5, :], nrow[:])
                        nc.vector.tensor_scalar(
                            out=sq[b:b + 3, :], in0=U[b:b + 3, :],
                            scalar1=-2.0, scalar2=None,
                            op0=mybir.AluOpType.mult)
                        nc.sync.dma_start(U[a:a + 3, :], sq[b:b + 3, :])

            # ---- main passes ----
            with tc.tile_pool(name="rowbuf", bufs=2) as rowbuf_pool, \
                 tc.tile_pool(name="mainpsum", bufs=2, space="PSUM") as psum_pool, \
                 tc.tile_pool(name="scratch", bufs=2) as scratch_pool:
                for _ in range(repeat):
                    _emit_pass(nc, U1, U2, 0, krows, d1t, i1t, rowbuf_pool,
                               psum_pool, scratch_pool, n_pts, m_pts, mm_dt)
                    _emit_pass(nc, U2, U1, 32, krows, d2t, i2t, rowbuf_pool,
                               psum_pool, scratch_pool, m_pts, n_pts, mm_dt)

            # clamp tiny negative rounding like the reference's max(d, 0)
            nc.scalar.activation(d1t[:], d1t[:], mybir.ActivationFunctionType.Relu)
            nc.scalar.activation(d2t[:], d2t[:], mybir.ActivationFunctionType.Relu)
            nc.sync.dma_start(d1.ap(), d1t[:])
            nc.sync.dma_start(i1.ap(), i1t[:])
            nc.sync.dma_start(d2.ap(), d2t[:])
            nc.sync.dma_start(i2.ap(), i2t[:])

    nc.compile()
    _PROGRAM_CACHE[key] = nc
    return nc


def _host_forms(xyz):
    """fp16 hi/lo split of the [N, 3] coords, laid out as the two 16-row
    matmul operand forms (norm rows zero-filled; the device computes and
    writes them)."""
    n = xyz.shape[0]
    co = np.ascontiguousarray(xyz.T)  # [3, n] f32
    hi = co.astype(np.float16)
    lo = (co - hi.astype(np.float32)).astype(np.float16)
    m2hi = (-2.0 * hi.astype(np.float32)).astype(np.float16)
    m2lo = (-2.0 * lo.astype(np.float32)).astype(np.float16)
    zero = np.zeros((1, n), np.float16)
    one = np.ones((1, n), np.float16)
    # staging piece list indexed by the _T_* constants
    P = [hi[0:1], hi[1:2], hi[2:3], lo[0:1], lo[1:2], lo[2:3],
         m2hi[0:1], m2hi[1:2], m2hi[2:3], m2lo[0:1], m2lo[1:2], m2lo[2:3],
         zero, zero, one]
    A = np.concatenate([P[t] for t in _A_ROWS], axis=0)
    Bf = np.concatenate([P[t] for t in _B_ROWS], axis=0)
    return A, Bf


def make_in_maps(xyz1, xyz2, mm_dtype="f16x2"):
    B, N, _ = xyz1.shape
    M = xyz2.shape[1]
    ones_n = np.ones((1, N), np.float32)
    ones_m = np.ones((1, M), np.float32)
    in_maps = []
    for b in range(B):
        m = {"c1": np.concatenate([np.ascontiguousarray(xyz1[b].T), ones_n]),
             "c2": np.concatenate([np.ascontiguousarray(xyz2[b].T), ones_m])}
        if mm_dtype == "f16x2":
            m["a1"], m["b1"] = _host_forms(xyz1[b])
            m["a2"], m["b2"] = _host_forms(xyz2[b])
        in_maps.append(m)
    return in_maps


def kernel(xyz1: np.ndarray, xyz2: np.ndarray, mm_dtype: str = "f16x2",
           repeat: int = 1, _return_results_only: bool = False):
    xyz1 = np.asarray(xyz1, dtype=np.float32)
    xyz2 = np.asarray(xyz2, dtype=np.float32)
    B, N, _ = xyz1.shape
    _, M, _ = xyz2.shape
    assert B == 8 and N == 8192 and M == 8192, (B, N, M)

    nc = _build_program(N, M, B, mm_dtype, repeat)
    in_maps = make_in_maps(xyz1, xyz2, mm_dtype)
    res = run_bass_kernel_spmd(nc, in_maps, list(range(B)))

    dist1 = np.empty((B, N), np.float32)
    dist2 = np.empty((B, M), np.float32)
    idx1 = np.empty((B, N), np.int32)
    idx2 = np.empty((B, M), np.int32)
    for b in range(B):
        r = res.results[b]
        dist1[b] = np.asarray(r["d1"]).T.reshape(-1)
        idx1[b] = np.asarray(r["i1"]).T.reshape(-1)
        dist2[b] = np.asarray(r["d2"]).T.reshape(-1)
        idx2[b] = np.asarray(r["i2"]).T.reshape(-1)
    return dist1, dist2, idx1, idx2


# revision 19
# speedup vs baseline: 3.3938x; 1.7643x over previous
"""Chamfer distance kernel for Trainium2 (8 NeuronCores, batch-parallel).

Strategy
--------
B=8 batches, one per core (SPMD: same program, per-core data).
Per core (N=M=8192, 3-D points):
  d[n,m] = |x1_n|^2 + |x2_m|^2 - 2 x1_n.x2_m  is computed fully inside
  PSUM by a single matmul over homogeneous-coordinate rows.  In the
  default "f16x2" mode every f32 operand row is split into an fp16
  (hi, lo) pair and the dot product expands into K=16 fp16 rows
  (4 per coordinate + 4 norm/ones rows): fp16 matmuls run 4x faster
  than f32 on the PE while the hi/lo split keeps ~22 mantissa bits, so
  the argmin matches the f32 reference.
  Two symmetric passes (rows = x1 points, then rows = x2 points) give
  the row-direction min+argmin for both outputs.  Per 128-row block the
  ACT engine moves PSUM->SBUF, DVE computes the row min with one
  full-row tensor_reduce straight into the dist column and one full-row
  max_index (first occurrence == np.argmin tie-break) writing the
  block's 8-wide slot of the widened index tile.
Outputs come back as [128, 64] dist tiles and [128, 512] index tiles
(partition-major); the host transpose-flattens them, keeping every 8th
index column.
"""

import numpy as np

import concourse.bacc as bacc
import concourse.bass as bass
import concourse.mybir as mybir
from concourse import tile
from concourse.bass_utils import run_bass_kernel_spmd

F32 = mybir.dt.float32
F16 = mybir.dt.float16
I32 = mybir.dt.int32
U32 = mybir.dt.uint32

_PROGRAM_CACHE = {}

# staging-row indices for the fp16 split pieces
_T_XH, _T_YH, _T_ZH = 0, 1, 2        # hi(x, y, z)
_T_XL, _T_YL, _T_ZL = 3, 4, 5        # lo(x, y, z)
_T_M2XH, _T_M2YH, _T_M2ZH = 6, 7, 8  # -2*hi
_T_M2XL, _T_M2YL, _T_M2ZL = 9, 10, 11  # -2*lo
_T_NH, _T_NL, _T_ONE = 12, 13, 14    # hi(n), lo(n), ones

# K=16 row contents of the two matmul operand forms, as staging indices.
# Pairing k: A_k . B_k summed over k gives n + n' - 2 x.x'.
_A_ROWS = [_T_M2XH, _T_M2XH, _T_M2XL, _T_M2XL,
           _T_M2YH, _T_M2YH, _T_M2YL, _T_M2YL,
           _T_M2ZH, _T_M2ZH, _T_M2ZL, _T_M2ZL,
           _T_ONE, _T_ONE, _T_NH, _T_NL]
_B_ROWS = [_T_XH, _T_XL, _T_XH, _T_XL,
           _T_YH, _T_YL, _T_YH, _T_YL,
           _T_ZH, _T_ZL, _T_ZH, _T_ZL,
           _T_NH, _T_NL, _T_ONE, _T_ONE]


def _emit_pass(nc, lhsU, rhsU, base, krows, dtile, itile, rowbuf_pool,
               psum_pool, scratch_pool, n_pts, m_pts, mm_dt):
    """One direction: for each 128-row block of lhs points, min+argmin over
    all m_pts columns.  lhsU[base:base+krows] is the stationary operand
    (A-form), rhsU[base:base+krows] the moving one (B-form); matmul
    requires both operands at the same partition base."""
    n_blocks = n_pts // 128
    n_groups = m_pts // 2048
    half = m_pts // 2

    for nb in range(n_blocks):
        rowbuf = rowbuf_pool.tile([128, m_pts], F32, tag="rowbuf")
        lhs_ap = lhsU[base:base + krows, nb * 128:(nb + 1) * 128]
        if mm_dt is not None:
            lhs_ap = lhs_ap.bitcast(mm_dt)
        for g in range(n_groups):
            psum = psum_pool.tile([128, 2048], F32, tag="psum")
            for q in range(4):
                rhs_ap = rhsU[base:base + krows,
                              (g * 4 + q) * 512:(g * 4 + q + 1) * 512]
                if mm_dt is not None:
                    rhs_ap = rhs_ap.bitcast(mm_dt)
                nc.tensor.matmul(
                    psum[:, q * 512:(q + 1) * 512],
                    lhs_ap,
                    rhs_ap,
                    start=True, stop=True,
                )
            nc.scalar.activation(
                rowbuf[:, g * 2048:(g + 1) * 2048], psum[:],
                mybir.ActivationFunctionType.Copy,
            )
        # row min -> dist column (full-row reduce; on HW this runs at the
        # same fast DVE rate as max_index, unlike the serialized scan)
        nc.vector.tensor_reduce(
            dtile[:, nb:nb + 1], rowbuf[:],
            axis=mybir.AxisListType.X, op=mybir.AluOpType.min,
        )
        # match the min value back to its first position; max_index writes
        # its 8-wide result straight into the block's slot of the widened
        # index tile (the host keeps every 8th column)
        q8 = scratch_pool.tile([128, 8], F32, tag="q8")
        nc.vector.tensor_copy(q8[:], dtile[:, nb:nb + 1].broadcast_to((128, 8)))
        nc.vector.max_index(itile[:, nb * 8:(nb + 1) * 8].bitcast(U32),
                            q8[:], rowbuf[:])


def _emit_prep_f16x2(nc, tc, U1, U2, c1, c2, a1, b1, a2, b2, n_pts, m_pts):
    """Load the host-prepared fp16 hi/lo coordinate forms and fill in the
    norm rows on device.  U1: A-form at base 0, B-form at base 32.  U2:
    B-form at 0, A-form at 32 (pass A pairs U1[0:16] x U2[0:16]; pass B
    pairs U2[32:48] x U1[32:48]).  Norm rows: A-form 14/15 = hi/lo(n),
    B-form 12/13 = hi/lo(n), computed from the f32 coords via the
    ones-matmul and an exact fp16 hi/lo split."""
    with tc.tile_pool(name="prep", bufs=1) as prep, \
         tc.tile_pool(name="preppsum", bufs=2, space="PSUM") as ppsum:
        ones_col = prep.tile([3, 1], F32, tag="ones_col")
        nc.vector.memset(ones_col[:], 1.0)
        for U, c, af, bf, npts, a, b in (
                (U1, c1, a1, b1, n_pts, 0, 32),
                (U2, c2, a2, b2, m_pts, 32, 0)):
            nc.sync.dma_start(U[a:a + 16, :], af.ap())
            nc.sync.dma_start(U[b:b + 16, :], bf.ap())
            S32 = prep.tile([3, npts], F32, tag="S32")
            SQ = prep.tile([3, npts], F32, tag="SQ")
            nrow = prep.tile([1, npts], F32, tag="nrow")
            NH32 = prep.tile([1, npts], F32, tag="NH32")
            NH = prep.tile([1, npts], F16, tag="NH")
            NL = prep.tile([1, npts], F16, tag="NL")

            nc.sync.dma_start(S32[:], c.ap()[0:3, :])  # x, y, z
            # norms in exact f32: n = x^2 + y^2 + z^2 via ones-matmul
            nc.scalar.activation(SQ[:], S32[:],
                                 mybir.ActivationFunctionType.Square)
            for cc in range(npts // 512):
                ps = ppsum.tile([1, 512], F32, tag="ps")
                nc.tensor.matmul(ps[:], ones_col[:],
                                 SQ[:, cc * 512:(cc + 1) * 512],
                                 start=True, stop=True)
                nc.scalar.activation(nrow[:, cc * 512:(cc + 1) * 512],
                                     ps[:], mybir.ActivationFunctionType.Copy)
            # exact hi/lo split of n: hi = fp16(n); lo = fp16(n - f32(hi))
            nc.scalar.activation(NH[:], nrow[:],
                                 mybir.ActivationFunctionType.Copy)
            nc.scalar.activation(NH32[:], NH[:],
                                 mybir.ActivationFunctionType.Copy)
            nc.vector.tensor_tensor(nrow[:], nrow[:], NH32[:],
                                    mybir.AluOpType.subtract)
            nc.scalar.activation(NL[:], nrow[:],
                                 mybir.ActivationFunctionType.Copy)
            for row, src in ((a + 14, NH), (a + 15, NL),
                             (b + 12, NH), (b + 13, NL)):
                nc.sync.dma_start(U[row:row + 1, :], src[:])


def _build_program(n_pts=8192, m_pts=8192, n_cores=8, mm_dtype="f16x2",
                   repeat=1):
    key = (n_pts, m_pts, n_cores, mm_dtype, repeat)
    if key in _PROGRAM_CACHE:
        return _PROGRAM_CACHE[key]

    nc = bacc.Bacc("TRN2", target_bir_lowering=False, debug=False,
                   num_devices=n_cores)
    # rows [x, y, z, ones]; ones seeds the homogeneous-coordinate rows
    c1 = nc.dram_tensor("c1", [4, n_pts], F32, kind="ExternalInput")
    c2 = nc.dram_tensor("c2", [4, m_pts], F32, kind="ExternalInput")
    if mm_dtype == "f16x2":
        a1 = nc.dram_tensor("a1", [16, n_pts], F16, kind="ExternalInput")
        b1 = nc.dram_tensor("b1", [16, n_pts], F16, kind="ExternalInput")
        a2 = nc.dram_tensor("a2", [16, m_pts], F16, kind="ExternalInput")
        b2 = nc.dram_tensor("b2", [16, m_pts], F16, kind="ExternalInput")
    d1 = nc.dram_tensor("d1", [128, n_pts // 128], F32, kind="ExternalOutput")
    i1 = nc.dram_tensor("i1", [128, n_pts // 16], I32, kind="ExternalOutput")
    d2 = nc.dram_tensor("d2", [128, m_pts // 128], F32, kind="ExternalOutput")
    i2 = nc.dram_tensor("i2", [128, m_pts // 16], I32, kind="ExternalOutput")

    with tile.TileContext(nc) as tc:
        with tc.tile_pool(name="persist", bufs=1) as persist, \
             tc.tile_pool(name="finals", bufs=1) as finals:
            d1t = finals.tile([128, n_pts // 128], F32, tag="d1t")
            i1t = finals.tile([128, n_pts // 16], I32, tag="i1t")
            d2t = finals.tile([128, m_pts // 128], F32, tag="d2t")
            i2t = finals.tile([128, m_pts // 16], I32, tag="i2t")

            if mm_dtype == "f16x2":
                krows, mm_dt = 16, None
                U1 = persist.tile([48, n_pts], F16, tag="U1")
                U2 = persist.tile([48, m_pts], F16, tag="U2")
                _emit_prep_f16x2(nc, tc, U1, U2, c1, c2, a1, b1, a2, b2,
                                 n_pts, m_pts)
            else:
                # exact-f32 fallback: K=5 homogeneous rows
                # U1: lhsT-form (A = [-2x,-2y,-2z, 1, n1]) at rows 0-4,
                #     rhs-form  (B = [x, y, z, n1, 1])     at rows 32-36
                # U2: rhs-form at rows 0-4, lhsT-form at rows 32-36
                krows, mm_dt = 5, None
                U1 = persist.tile([37, n_pts], F32, tag="U1")
                U2 = persist.tile([37, m_pts], F32, tag="U2")
                ones_col = persist.tile([35, 1], F32, tag="ones_col")
                nc.vector.memset(ones_col[:], 1.0)
                with tc.tile_pool(name="prep", bufs=1) as prep, \
                     tc.tile_pool(name="preppsum", bufs=2, space="PSUM") as ppsum:
                    for U, c, npts, a, b in ((U1, c1, n_pts, 0, 32),
                                             (U2, c2, m_pts, 32, 0)):
                        nc.sync.dma_start(U[b:b + 3, :], c.ap()[0:3, :])
                        nc.sync.dma_start(U[a + 3:a + 4, :], c.ap()[3:4, :])
                        nc.sync.dma_start(U[b + 4:b + 5, :], c.ap()[3:4, :])
                        sq = prep.tile([35, npts], F32, tag="sq")
                        nrow = prep.tile([1, npts], F32, tag="nrow")
                        nc.scalar.activation(sq[b:b + 3, :], U[b:b + 3, :],
                                             mybir.ActivationFunctionType.Square)
                        for cc in range(npts // 512):
                            ps = ppsum.tile([1, 512], F32, tag="ps")
                            nc.tensor.matmul(ps[:], ones_col[b:b + 3, :],
                                             sq[b:b + 3, cc * 512:(cc + 1) * 512],
                                             start=True, stop=True)
                            nc.scalar.activation(
                                nrow[:, cc * 512:(cc + 1) * 512],
                                ps[:], mybir.ActivationFunctionType.Copy)
                        nc.sync.dma_start(U[b + 3:b + 4, :], nrow[:])
                        nc.sync.dma_start(U[a + 4:a + 5, :], nrow[:])
                        nc.vector.tensor_scalar(
                            out=sq[b:b + 3, :], in0=U[b:b + 3, :],
                            scalar1=-2.0, scalar2=None,
                            op0=mybir.AluOpType.mult)
                        nc.sync.dma_start(U[a:a + 3, :], sq[b:b + 3, :])

            # ---- main passes ----
            with tc.tile_pool(name="rowbuf", bufs=3) as rowbuf_pool, \
                 tc.tile_pool(name="mainpsum", bufs=2, space="PSUM") as psum_pool, \
                 tc.tile_pool(name="scratch", bufs=2) as scratch_pool:
                for _ in range(repeat):
                    _emit_pass(nc, U1, U2, 0, krows, d1t, i1t, rowbuf_pool,
                               psum_pool, scratch_pool, n_pts, m_pts, mm_dt)
                    _emit_pass(nc, U2, U1, 32, krows, d2t, i2t, rowbuf_pool,
                               psum_pool, scratch_pool, m_pts, n_pts, mm_dt)

            # clamp tiny negative rounding like the reference's max(d, 0)
            nc.scalar.activation(d1t[:], d1t[:], mybir.ActivationFunctionType.Relu)
            nc.scalar.activation(d2t[:], d2t[:], mybir.ActivationFunctionType.Relu)
            nc.sync.dma_start(d1.ap(), d1t[:])
            nc.sync.dma_start(i1.ap(), i1t[:])
            nc.sync.dma_start(d2.ap(), d2t[:])
            nc.sync.dma_start(i2.ap(), i2t[:])

    nc.compile()
    _PROGRAM_CACHE[key] = nc
    return nc


def _host_forms(xyz):
    """fp16 hi/lo split of the [N, 3] coords, laid out as the two 16-row
    matmul operand forms (norm rows zero-filled; the device computes and
    writes them)."""
    n = xyz.shape[0]
    co = np.ascontiguousarray(xyz.T)  # [3, n] f32
    hi = co.astype(np.float16)
    lo = (co - hi.astype(np.float32)).astype(np.float16)
    m2hi = (-2.0 * hi.astype(np.float32)).astype(np.float16)
    m2lo = (-2.0 * lo.astype(np.float32)).astype(np.float16)
    zero = np.zeros((1, n), np.float16)
    one = np.ones((1, n), np.float16)
    # staging piece list indexed by the _T_* constants
    P = [hi[0:1], hi[1:2], hi[2:3], lo[0:1], lo[1:2], lo[2:3],
         m2hi[0:1], m2hi[1:2], m2hi[2:3], m2lo[0:1], m2lo[1:2], m2lo[2:3],
         zero, zero, one]
    A = np.concatenate([P[t] for t in _A_ROWS], axis=0)
    Bf = np.concatenate([P[t] for t in _B_ROWS], axis=0)
    return A, Bf


def make_in_maps(xyz1, xyz2, mm_dtype="f16x2"):
    B, N, _ = xyz1.shape
    M = xyz2.shape[1]
    ones_n = np.ones((1, N), np.float32)
    ones_m = np.ones((1, M), np.float32)
    in_maps = []
    for b in range(B):
        m = {"c1": np.concatenate([np.ascontiguousarray(xyz1[b].T), ones_n]),
             "c2": np.concatenate([np.ascontiguousarray(xyz2[b].T), ones_m])}
        if mm_dtype == "f16x2":
            m["a1"], m["b1"] = _host_forms(xyz1[b])
            m["a2"], m["b2"] = _host_forms(xyz2[b])
        in_maps.append(m)
    return in_maps


def kernel(xyz1: np.ndarray, xyz2: np.ndarray, mm_dtype: str = "f16x2",
           repeat: int = 1, _return_results_only: bool = False):
    xyz1 = np.asarray(xyz1, dtype=np.float32)
    xyz2 = np.asarray(xyz2, dtype=np.float32)
    B, N, _ = xyz1.shape
    _, M, _ = xyz2.shape
    assert B == 8 and N == 8192 and M == 8192, (B, N, M)

    nc = _build_program(N, M, B, mm_dtype, repeat)
    in_maps = make_in_maps(xyz1, xyz2, mm_dtype)
    res = run_bass_kernel_spmd(nc, in_maps, list(range(B)))

    dist1 = np.empty((B, N), np.float32)
    dist2 = np.empty((B, M), np.float32)
    idx1 = np.empty((B, N), np.int32)
    idx2 = np.empty((B, M), np.int32)
    for b in range(B):
        r = res.results[b]
        dist1[b] = np.asarray(r["d1"]).T.reshape(-1)
        idx1[b] = np.asarray(r["i1"]).reshape(128, -1, 8)[:, :, 0].T.reshape(-1)
        dist2[b] = np.asarray(r["d2"]).T.reshape(-1)
        idx2[b] = np.asarray(r["i2"]).reshape(128, -1, 8)[:, :, 0].T.reshape(-1)
    return dist1, dist2, idx1, idx2
